# revision 1
# baseline (speedup 1.0000x reference)
"""Trainium2 Bass kernel for PointConv-style e3nn message passing.

Self-contained: builds + runs an 8-core SPMD Bass kernel via
bass_utils.run_bass_kernel_spmd, accepting FULL inputs and returning the
FULL output.

Sharding: nodes are padded to 20480 and split into 8 slices of 2560; edges
are bucketed by destination-node window (128 nodes per window, 20 windows
per core) so the scatter-add becomes per-window one-hot matmuls with no
inter-core communication.  Node features and weights are replicated.
"""

import os
import sys
import types
import ctypes

import numpy as np

import concourse.bass as bass
import concourse.bacc as bacc
import concourse.tile as tile
from concourse import mybir
from concourse.bass import AP, IndirectOffsetOnAxis
from concourse.bass_utils import run_bass_kernel_spmd
from concourse.masks import make_identity

# ---------------------------------------------------------------- constants
N = 20000
E = 160000
MUL = 64
EDIM = 8
NZ = 4
AVG_NEIGH = 8.0
INV_SQRT3 = float(1.0 / np.sqrt(3.0))

CORES = 8
NP_PAD = 20480            # padded node count (divisible by 128*8)
NPC = NP_PAD // CORES     # 2560 nodes per core
WIN = 128                 # nodes per scatter window
NWIN = NPC // WIN         # 20 windows per core
NGRP = NPC // 512         # 5 node groups of 512 per core

F32 = mybir.dt.float32
BF16 = mybir.dt.bfloat16
I32 = mybir.dt.int32
NP_BF16 = mybir.dt.np(mybir.dt.bfloat16)

LAST_RESULT = None        # BassKernelResults of the most recent run


# ------------------------------------------------------- axon profile hook
def _install_profile_hook():
    """Make trace=True / BASS_TRACE=1 work under axon (degrades silently)."""
    if "antenv.axon_hooks" in sys.modules:
        return
    try:
        try:
            from trn_agent_boot.trn_boot import _ntff_profile_via_ctypes
        except ImportError:
            sys.path.insert(0, "/root/.axon_site")
            from trn_agent_boot.trn_boot import _ntff_profile_via_ctypes
        so_path = "/opt/axon/libaxon_pjrt.so"
        lib = ctypes.CDLL(so_path)
        if not hasattr(lib, "axon_start_nrt_profile"):
            return
        hook = _ntff_profile_via_ctypes(so_path)
        mod = types.ModuleType("antenv.axon_hooks")
        state = {"hook": hook}
        mod.set_axon_ntff_profile_hook = lambda h: state.__setitem__("hook", h)
        mod.get_axon_ntff_profile_hook = lambda: state["hook"]
        sys.modules["antenv.axon_hooks"] = mod
        import antenv
        antenv.axon_hooks = mod
    except Exception:
        pass


# ----------------------------------------------- tile-exit drain workaround
def _patch_tile_drain():
    """This toolchain's walrus rejects >1 sem wait on a Drain; hang the exit
    waits on a NoOp chain instead (bacc's generate_event_semaphores then
    legalises them)."""
    from concourse.vector_clock import ScopedClock

    def _drain_and_barrier(self, tick_clock, wait_clock):
        nop_inst = self.nc.sync.nop(nofuse=True, hint="tile_exit_wait")
        wait_clock.add_sem_waits(
            nop_inst.ins, ScopedClock({None: tick_clock.global_clock})
        )
        self.nc.sync.drain()
        self.nc.all_engine_barrier()
        assert self.sems is not None
        popped = self.nc._tile_sem_poison_stack.pop()
        assert popped is self._sem_poison
        self.nc.clear_and_free_semaphores(list(self.sems.allocated().values()))
        self.nc.all_engine_barrier()

    tile.TileContext._drain_and_barrier = _drain_and_barrier


_patch_tile_drain()


def _apv(base_ap, col_off, dims):
    """AP view of a 2D sbuf/psum tile: partitions x custom free dims.

    dims: list of [step, count] free-dim pairs (element units).
    """
    pstep, pcount = base_ap.ap[0]
    return AP(base_ap.tensor, base_ap.offset + col_off, [[pstep, pcount]] + dims)


# ---------------------------------------------------------------- program
def _build_program(ET):
    """Build the SPMD Bass program; ET = edge tiles (128 edges) per window."""
    C = ET * 128  # edge capacity per window

    nc = bacc.Bacc()

    # inputs (per core)
    xTf = nc.dram_tensor("xTf", [256, NP_PAD], BF16, kind="ExternalInput")
    xT = nc.dram_tensor("xT", [256, NPC], F32, kind="ExternalInput")
    arep = nc.dram_tensor("arep", [256, NPC], F32, kind="ExternalInput")
    srcw = nc.dram_tensor("srcw", [NWIN, 128, ET], I32, kind="ExternalInput")
    dstw = nc.dram_tensor("dstw", [NWIN, 128, ET], BF16, kind="ExternalInput")
    a0w_d = nc.dram_tensor("a0w", [NWIN, 128, ET], F32, kind="ExternalInput")
    a1w_d = nc.dram_tensor("a1w", [NWIN, 128, 3 * ET], BF16, kind="ExternalInput")
    embw_d = nc.dram_tensor("embw", [NWIN, EDIM, C], BF16, kind="ExternalInput")
    wblk1_d = nc.dram_tensor("wblk1", [128, 128], BF16, kind="ExternalInput")
    wblk2_d = nc.dram_tensor("wblk2", [128, 128], BF16, kind="ExternalInput")
    wm1_d = nc.dram_tensor("wm1", [EDIM, EDIM], BF16, kind="ExternalInput")
    wbig_d = nc.dram_tensor("wbig", [EDIM, 256], BF16, kind="ExternalInput")
    w20_d = nc.dram_tensor("w20", [128, 128], BF16, kind="ExternalInput")
    w21_d = nc.dram_tensor("w21", [128, 64], BF16, kind="ExternalInput")
    wsc0_d = nc.dram_tensor("wsc0", [2, 128, 128], BF16, kind="ExternalInput")
    wsc1_d = nc.dram_tensor("wsc1", [2, 128, 64], BF16, kind="ExternalInput")
    iota_d = nc.dram_tensor("iota", [128, 128], BF16, kind="ExternalInput")
    outT = nc.dram_tensor("outT", [256, NPC], F32, kind="ExternalOutput")
    debug = bool(int(os.environ.get("BASS_DEBUG_DUMP", "0")))
    if debug:
        stdump = nc.dram_tensor("stdump", [NWIN, 512, 128], F32,
                                kind="ExternalOutput")

    ACT_SILU = mybir.ActivationFunctionType.Silu
    ACT_COPY = mybir.ActivationFunctionType.Copy
    MULT = mybir.AluOpType.mult
    ISEQ = mybir.AluOpType.is_equal

    with tile.TileContext(nc) as tc:
        with (
            tc.tile_pool(name="const", bufs=1) as cp,
            tc.tile_pool(name="win", bufs=2) as wpool,
            tc.tile_pool(name="et", bufs=2) as ep,
            tc.tile_pool(name="sts", bufs=2) as sp,
            tc.tile_pool(name="node", bufs=2) as npool,
            tc.tile_pool(name="psum", bufs=2, space="PSUM") as pp,
            tc.tile_pool(name="dram", bufs=1, space="DRAM") as dp,
            tc.tile_pool(name="hstage", bufs=2) as hp_pool,
        ):
            hdram = dp.tile([NP_PAD, 256], BF16)
            # ---- constants
            iota = cp.tile([128, 128], BF16)
            nc.sync.dma_start(out=iota[:], in_=iota_d[:])
            wblk1 = cp.tile([128, 128], BF16)
            nc.sync.dma_start(out=wblk1[:], in_=wblk1_d[:])
            wblk2 = cp.tile([128, 128], BF16)
            nc.sync.dma_start(out=wblk2[:], in_=wblk2_d[:])
            wm1 = cp.tile([EDIM, EDIM], BF16)
            nc.sync.dma_start(out=wm1[:], in_=wm1_d[:])
            wbig = cp.tile([EDIM, 256], BF16)
            nc.sync.dma_start(out=wbig[:], in_=wbig_d[:])
            w20 = cp.tile([128, 128], BF16)
            nc.sync.dma_start(out=w20[:], in_=w20_d[:])
            w21 = cp.tile([128, 64], BF16)
            nc.sync.dma_start(out=w21[:], in_=w21_d[:])
            wsc0a = cp.tile([128, 128], BF16)
            nc.sync.dma_start(out=wsc0a[:], in_=wsc0_d[0])
            wsc0b = cp.tile([128, 128], BF16)
            nc.sync.dma_start(out=wsc0b[:], in_=wsc0_d[1])
            wsc1a = cp.tile([128, 64], BF16)
            nc.sync.dma_start(out=wsc1a[:], in_=wsc1_d[0])
            wsc1b = cp.tile([128, 64], BF16)
            nc.sync.dma_start(out=wsc1b[:], in_=wsc1_d[1])

            sts = []  # per-chunk sT sbuf tiles [128 chan, 512 nodes]
            for k in range(4):
                sts.append(None)

            def node_phase(g):
                """Finish nodes [g*512, (g+1)*512) of this core's slice."""
                cols = slice(g * 512, (g + 1) * 512)
                # x slices (channel-major)
                xga = npool.tile([128, 512], F32, tag="xga")
                nc.sync.dma_start(out=xga[:], in_=xT[0:128, cols])
                xgb = npool.tile([128, 512], F32, tag="xgb")
                nc.sync.dma_start(out=xgb[:], in_=xT[128:256, cols])
                ara = npool.tile([128, 512], F32, tag="ara")
                nc.sync.dma_start(out=ara[:], in_=arep[0:128, cols])
                arb = npool.tile([128, 512], F32, tag="arb")
                nc.sync.dma_start(out=arb[:], in_=arep[128:256, cols])

                # u0 + sc0 -> up0 [128 rows: scalars|gates, 512 nodes]
                up0 = pp.tile([128, 512], F32, tag="npsum", bufs=2)
                nc.tensor.matmul(up0[:], lhsT=w20[:], rhs=sts[0][:],
                                 start=True, stop=False)
                # y = x0 (x4-replicated) * attrs_rep ; via DMA-replicated x0
                x4 = npool.tile([128, 512], F32, tag="x4")
                nc.sync.dma_start(out=x4[0:64, :], in_=xT[0:64, cols])
                nc.sync.dma_start(out=x4[64:128, :], in_=xT[0:64, cols])
                ya = npool.tile([128, 512], BF16, tag="ya")
                nc.vector.tensor_tensor(out=ya[:], in0=x4[:], in1=ara[:], op=MULT)
                yb = npool.tile([128, 512], BF16, tag="yb")
                nc.vector.tensor_tensor(out=yb[:], in0=x4[:], in1=arb[:], op=MULT)
                nc.tensor.matmul(up0[:], lhsT=wsc0a[:], rhs=ya[:],
                                 start=False, stop=False)
                nc.tensor.matmul(up0[:], lhsT=wsc0b[:], rhs=yb[:],
                                 start=False, stop=True)

                # u1 + sc1 per d -> up1a rows[0:64]=d0 rows[64:128]=d1; up1b=d2
                up1a = pp.tile([128, 512], F32, tag="npsum", bufs=2)
                nc.tensor.matmul(up1a[0:64, :], lhsT=w21[:], rhs=sts[1][:],
                                 start=True, stop=False)
                nc.tensor.matmul(up1a[64:128, :], lhsT=w21[:], rhs=sts[2][:],
                                 start=True, stop=False)
                for d in (0, 1):
                    x4d = npool.tile([128, 512], F32, tag="x4d")
                    nc.sync.dma_start(
                        out=x4d[0:64, :],
                        in_=xT[64 + 64 * d:128 + 64 * d, cols])
                    nc.sync.dma_start(
                        out=x4d[64:128, :],
                        in_=xT[64 + 64 * d:128 + 64 * d, cols])
                    yda = npool.tile([128, 512], BF16, tag="yda")
                    nc.vector.tensor_tensor(out=yda[:], in0=x4d[:], in1=ara[:], op=MULT)
                    ydb = npool.tile([128, 512], BF16, tag="ydb")
                    nc.vector.tensor_tensor(out=ydb[:], in0=x4d[:], in1=arb[:], op=MULT)
                    rows = slice(64 * d, 64 * d + 64)
                    nc.tensor.matmul(up1a[rows, :], lhsT=wsc1a[:], rhs=yda[:],
                                     start=False, stop=False)
                    nc.tensor.matmul(up1a[rows, :], lhsT=wsc1b[:], rhs=ydb[:],
                                     start=False, stop=True)

                # gate scalars/gates
                t0s = npool.tile([128, 512], F32, tag="t0s")
                nc.scalar.activation(t0s[:], up0[:], ACT_SILU)

                # d2 into its own psum slot (after up0 freed)
                up1b = pp.tile([64, 512], F32, tag="npsum", bufs=2)
                nc.tensor.matmul(up1b[:], lhsT=w21[:], rhs=sts[3][:],
                                 start=True, stop=False)
                x4d2 = npool.tile([128, 512], F32, tag="x4d")
                nc.sync.dma_start(out=x4d2[0:64, :], in_=xT[192:256, cols])
                nc.sync.dma_start(out=x4d2[64:128, :], in_=xT[192:256, cols])
                yda2 = npool.tile([128, 512], BF16, tag="yda")
                nc.vector.tensor_tensor(out=yda2[:], in0=x4d2[:], in1=ara[:], op=MULT)
                ydb2 = npool.tile([128, 512], BF16, tag="ydb")
                nc.vector.tensor_tensor(out=ydb2[:], in0=x4d2[:], in1=arb[:], op=MULT)
                nc.tensor.matmul(up1b[:], lhsT=wsc1a[:], rhs=yda2[:],
                                 start=False, stop=False)
                nc.tensor.matmul(up1b[:], lhsT=wsc1b[:], rhs=ydb2[:],
                                 start=False, stop=True)

                # vectors = gates * t1_d ; resnet add ; assemble outT rows
                outa = npool.tile([128, 512], F32, tag="outa")
                nc.vector.tensor_add(out=outa[0:64, :], in0=t0s[0:64, :],
                                     in1=xga[0:64, :])
                nc.vector.tensor_tensor(out=outa[64:128, :], in0=t0s[64:128, :],
                                        in1=up1a[0:64, :], op=MULT)
                nc.vector.tensor_add(out=outa[64:128, :], in0=outa[64:128, :],
                                     in1=xga[64:128, :])
                outb = npool.tile([128, 512], F32, tag="outb")
                nc.vector.tensor_tensor(out=outb[0:64, :], in0=t0s[64:128, :],
                                        in1=up1a[64:128, :], op=MULT)
                nc.vector.tensor_add(out=outb[0:64, :], in0=outb[0:64, :],
                                     in1=xgb[0:64, :])
                nc.vector.tensor_tensor(out=outb[64:128, :], in0=t0s[64:128, :],
                                        in1=up1b[:], op=MULT)
                nc.vector.tensor_add(out=outb[64:128, :], in0=outb[64:128, :],
                                     in1=xgb[64:128, :])
                nc.sync.dma_start(out=outT[0:128, cols], in_=outa[:])
                nc.sync.dma_start(out=outT[128:256, cols], in_=outb[:])

            # ================= phase H: h table =================
            for jj in range(0, NP_PAD, 2048):
                xa = hp_pool.tile([128, 2048], BF16, tag="xa")
                nc.sync.dma_start(out=xa[:], in_=xTf[0:128, jj:jj + 2048])
                xb = hp_pool.tile([128, 2048], BF16, tag="xb")
                nc.sync.dma_start(out=xb[:], in_=xTf[128:256, jj:jj + 2048])
                for t4 in range(16):
                    nt = jj + 128 * t4
                    sl = slice(128 * t4, 128 * t4 + 128)
                    hpp = pp.tile([128, 256], F32, tag="scrm", bufs=2)
                    nc.tensor.matmul(hpp[:, 0:128], lhsT=xa[:, sl],
                                     rhs=wblk1[:], start=True, stop=True)
                    nc.tensor.matmul(hpp[:, 128:256], lhsT=xb[:, sl],
                                     rhs=wblk2[:], start=True, stop=True)
                    hsb = hp_pool.tile([128, 256], BF16, tag="hsb", bufs=3)
                    if t4 % 2 == 0:
                        nc.scalar.activation(hsb[:], hpp[:], ACT_COPY)
                    else:
                        nc.vector.tensor_copy(out=hsb[:], in_=hpp[:])
                    nc.sync.dma_start(out=hdram[nt:nt + 128, :], in_=hsb[:])

            # ================= main loop over windows =================
            for w in range(NWIN):
                if w % 4 == 0:
                    for k in range(4):
                        sts[k] = sp.tile([128, 512], BF16, tag=f"sts{k}",
                                         name=f"sts{k}_{w // 4}")
                wcol = (w % 4) * 128

                sit = wpool.tile([128, ET], I32, tag="sit")
                nc.sync.dma_start(out=sit[:], in_=srcw[w])
                dot = wpool.tile([128, ET], BF16, tag="dot")
                nc.sync.dma_start(out=dot[:], in_=dstw[w])
                a0t = wpool.tile([128, ET], F32, tag="a0t")
                nc.sync.dma_start(out=a0t[:], in_=a0w_d[w])
                a1t = wpool.tile([128, 3 * ET], BF16, tag="a1t")
                nc.sync.dma_start(out=a1t[:], in_=a1w_d[w])
                emw = wpool.tile([EDIM, C], BF16, tag="emw")
                nc.sync.dma_start(out=emw[:], in_=embw_d[w])

                # radial MLP hidden: hidT[j, e] = silu((emb @ Wm1s).T)
                hid = wpool.tile([EDIM, C], BF16, tag="hid")
                for c0 in range(0, C, 512):
                    sz = min(512, C - c0)
                    hp = pp.tile([EDIM, 512], F32, tag="scrm", bufs=2)
                    nc.tensor.matmul(hp[:, :sz], lhsT=wm1[:],
                                     rhs=emw[:, c0:c0 + sz],
                                     start=True, stop=True)
                    nc.scalar.activation(hid[:, c0:c0 + sz], hp[:, :sz],
                                         ACT_SILU)

                stp = pp.tile([128, 512], F32, tag="stp", bufs=2)
                nc.vector.memset(stp[:], 0.0)
                for t in range(ET):
                    # gather h rows of the edge sources
                    hs = ep.tile([128, 256], BF16, tag="hs", bufs=3)
                    nc.gpsimd.indirect_dma_start(
                        out=hs[:], out_offset=None, in_=hdram[:],
                        in_offset=IndirectOffsetOnAxis(ap=sit[:, t:t + 1],
                                                       axis=0))

                    # per-edge tp weights: wp = hid_t.T @ WBIG  [128e, 256]
                    wp = pp.tile([128, 256], F32, tag="wp", bufs=2)
                    nc.tensor.matmul(wp[:], lhsT=hid[:, 128 * t:128 * t + 128],
                                     rhs=wbig[:], start=True, stop=True)

                    # wg = [w1g0|w2g0 | w3g1(d-major)|w4g1(d-major)]
                    wg = ep.tile([128, 512], BF16, tag="wg")
                    nc.vector.tensor_tensor(
                        out=_apv(wg[:], 0, [[64, 2], [1, 64]]),
                        in0=_apv(wp[:], 0, [[64, 2], [1, 64]]),
                        in1=_apv(hs[:], 0, [[0, 2], [1, 64]]),
                        op=MULT)
                    nc.vector.tensor_tensor(
                        out=_apv(wg[:], 128, [[192, 2], [64, 3], [1, 64]]),
                        in0=_apv(wp[:], 128, [[64, 2], [0, 3], [1, 64]]),
                        in1=_apv(hs[:], 64, [[0, 2], [64, 3], [1, 64]]),
                        op=MULT)

                    # messages [128e, 512]: chunks [m0a|m0b][m1a_d|m1b_d]*3
                    msg = ep.tile([128, 512], BF16, tag="msg")
                    nc.scalar.activation(
                        _apv(msg[:], 64, [[128, 4], [1, 64]]),
                        _apv(wg[:], 64, [[64, 4], [1, 64]]),
                        ACT_COPY, scale=a0t[:, t:t + 1])
                    nc.vector.tensor_tensor(
                        out=_apv(msg[:], 128, [[128, 3], [1, 64]]),
                        in0=_apv(wg[:], 0, [[0, 3], [1, 64]]),
                        in1=_apv(a1t[:], 3 * t, [[1, 3], [0, 64]]),
                        op=MULT)
                    tmp = ep.tile([128, 192], BF16, tag="tmp")
                    nc.vector.tensor_tensor(
                        out=_apv(tmp[:], 0, [[64, 3], [1, 64]]),
                        in0=_apv(wg[:], 320, [[64, 3], [1, 64]]),
                        in1=_apv(a1t[:], 3 * t, [[1, 3], [0, 64]]),
                        op=MULT)
                    nc.vector.tensor_add(out=msg[:, 0:64], in0=tmp[:, 0:64],
                                         in1=tmp[:, 64:128])
                    nc.vector.tensor_add(out=msg[:, 0:64], in0=msg[:, 0:64],
                                         in1=tmp[:, 128:192])

                    # one-hot of dst offsets, then segment matmuls
                    onehot = ep.tile([128, 128], BF16, tag="onehot")
                    nc.vector.tensor_tensor(
                        out=onehot[:],
                        in0=dot[:, t:t + 1].to_broadcast((128, 128)),
                        in1=iota[:], op=ISEQ)
                    # PSUM start=True clears has_written bits bank-wide, so
                    # four interleaved chunk chains can't each use it; memset
                    # the bank once (above) and accumulate with start=False.
                    for k in range(4):
                        nc.tensor.matmul(
                            stp[:, 128 * k:128 * k + 128],
                            lhsT=msg[:, 128 * k:128 * k + 128],
                            rhs=onehot[:],
                            start=False, stop=(t == ET - 1),
                            skip_group_check=True)

                # bank the window's sums into the group tiles
                for k in range(4):
                    nc.scalar.activation(sts[k][:, wcol:wcol + 128],
                                         stp[:, 128 * k:128 * k + 128],
                                         ACT_COPY)
                    if debug:
                        nc.sync.dma_start(
                            out=stdump[w, 128 * k:128 * k + 128, :],
                            in_=sts[k][:, wcol:wcol + 128])

                if w % 4 == 3:
                    node_phase(w // 4)

    nc.compile()
    return nc


# ---------------------------------------------------------------- host prep
def _host_prep(node_feats, node_attrs, edge_attrs, edge_embedding,
               W_lin1_0, W_lin1_1, W_mlp1, W_mlp2,
               W_lin2_0, W_lin2_1, W_sc0, W_sc1, edge_index):
    inv = 1.0 / np.sqrt(MUL)
    inv_e = 1.0 / np.sqrt(EDIM)
    inv2 = 1.0 / np.sqrt(2 * MUL)
    inv_n = 1.0 / np.sqrt(AVG_NEIGH)
    inv_sc = 1.0 / np.sqrt(MUL * NZ)

    # channel permutation: ours = [x0(64) | x1 d-major(192)]
    gidx = np.empty(256, np.int64)
    gidx[:64] = np.arange(64)
    for d in range(3):
        for u in range(64):
            gidx[64 + 64 * d + u] = 64 + 3 * u + d

    xgf = np.zeros((NP_PAD, 256), np.float32)
    xgf[:N] = node_feats[:, gidx]
    xT = np.ascontiguousarray(xgf.T)
    xTf = xT.astype(NP_BF16)

    arep_full = np.zeros((256, NP_PAD), np.float32)
    arep_full[:, :N] = np.repeat(node_attrs.T.astype(np.float32), MUL, axis=0)

    # ---- edge bucketing by destination window
    src = edge_index[0].astype(np.int64)
    dst = edge_index[1].astype(np.int64)
    wid = dst // WIN                      # global window id, 0..159
    order = np.argsort(wid, kind="stable")
    src_s, dst_s, wid_s = src[order], dst[order], wid[order]
    ea_s = edge_attrs[order].astype(np.float32)
    emb_s = edge_embedding[order].astype(np.float32)

    nwin_g = CORES * NWIN
    counts = np.bincount(wid_s, minlength=nwin_g)
    C = int(np.ceil(max(int(counts.max()), 1) / 128.0) * 128)
    ET = C // 128
    starts = np.zeros(nwin_g + 1, np.int64)
    np.cumsum(counts, out=starts[1:])

    per_core = []
    for c in range(CORES):
        srcw = np.zeros((NWIN, 128, ET), np.int32)
        dstw = np.full((NWIN, 128, ET), 1.0e6, np.float32)  # cast later
        a0w = np.zeros((NWIN, 128, ET), np.float32)
        a1w = np.zeros((NWIN, 128, 3 * ET), np.float32)
        embw = np.zeros((NWIN, EDIM, C), np.float32)
        for wl in range(NWIN):
            gw = c * NWIN + wl
            s, e = starts[gw], starts[gw + 1]
            n = e - s
            if n == 0:
                continue
            sl = slice(s, e)
            # slot j = t*128 + p  ->  (p, t)
            j = np.arange(n)
            p, t = j % 128, j // 128
            srcw[wl, p, t] = src_s[sl]
            dstw[wl, p, t] = (dst_s[sl] - gw * WIN).astype(np.float32)
            a0w[wl, p, t] = ea_s[sl, 0]
            for k in range(3):
                a1w[wl, p, 3 * t + k] = ea_s[sl, 1 + k]
            embw[wl, :, j] = emb_s[sl]
        per_core.append(dict(srcw=srcw, dstw=dstw.astype(NP_BF16),
                             a0w=a0w, a1w=a1w.astype(NP_BF16),
                             embw=embw.astype(NP_BF16)))

    # ---- weights
    W10s = (W_lin1_0 * inv).astype(np.float32)
    W11s = (W_lin1_1 * inv).astype(np.float32)
    wblk1 = np.zeros((128, 128), np.float32)
    wblk1[:64, :64] = W10s
    wblk1[64:, 64:] = W11s
    wblk2 = np.zeros((128, 128), np.float32)
    wblk2[:64, :64] = W11s
    wblk2[64:, 64:] = W11s
    wblk1 = wblk1.astype(NP_BF16)
    wblk2 = wblk2.astype(NP_BF16)
    wm1 = (W_mlp1 * inv_e).astype(NP_BF16)
    w1 = W_mlp2[:, 0:64]
    w2 = W_mlp2[:, 64:128]
    w3 = W_mlp2[:, 128:192]
    w4 = W_mlp2[:, 192:256]
    wbig = (np.concatenate([w2, w1, w3, w4 * INV_SQRT3], axis=1)
            * inv_e).astype(NP_BF16)
    w20 = (np.concatenate([W_lin2_0[64:128], W_lin2_0[0:64]])
           * inv2 * inv_n).astype(NP_BF16)
    w21 = (W_lin2_1 * inv2 * inv_n).astype(NP_BF16)
    wsc0z = (np.transpose(W_sc0, (1, 0, 2)).reshape(NZ * MUL, 2 * MUL)
             * inv_sc).astype(NP_BF16)
    wsc1z = (np.transpose(W_sc1, (1, 0, 2)).reshape(NZ * MUL, MUL)
             * inv_sc).astype(NP_BF16)
    wsc0 = np.stack([wsc0z[:128], wsc0z[128:]])
    wsc1 = np.stack([wsc1z[:128], wsc1z[128:]])
    iota = np.broadcast_to(np.arange(128, dtype=np.float32)[None, :],
                           (128, 128)).astype(NP_BF16)

    shared = dict(xTf=xTf, wblk1=wblk1, wblk2=wblk2, wm1=wm1,
                  wbig=wbig, w20=w20, w21=w21, wsc0=wsc0, wsc1=wsc1,
                  iota=iota)
    in_maps = []
    for c in range(CORES):
        m = dict(shared)
        m["xT"] = np.ascontiguousarray(xT[:, c * NPC:(c + 1) * NPC])
        m["arep"] = np.ascontiguousarray(arep_full[:, c * NPC:(c + 1) * NPC])
        m.update(per_core[c])
        in_maps.append(m)
    return ET, in_maps, gidx


_PROGRAM_CACHE = {}


def kernel(**inputs):
    global LAST_RESULT
    _install_profile_hook()

    args = {k: np.asarray(v) for k, v in inputs.items()}
    ET, in_maps, gidx = _host_prep(
        args["node_feats"].astype(np.float32),
        args["node_attrs"].astype(np.float32),
        args["edge_attrs"].astype(np.float32),
        args["edge_embedding"].astype(np.float32),
        args["W_lin1_0"].astype(np.float32),
        args["W_lin1_1"].astype(np.float32),
        args["W_mlp1"].astype(np.float32),
        args["W_mlp2"].astype(np.float32),
        args["W_lin2_0"].astype(np.float32),
        args["W_lin2_1"].astype(np.float32),
        args["W_sc0"].astype(np.float32),
        args["W_sc1"].astype(np.float32),
        args["edge_index"])

    if ET not in _PROGRAM_CACHE:
        _PROGRAM_CACHE[ET] = _build_program(ET)
    nc = _PROGRAM_CACHE[ET]

    trace = bool(int(os.environ.get("BASS_TRACE", "0")))
    res = run_bass_kernel_spmd(nc, in_maps, core_ids=list(range(CORES)),
                               trace=trace)
    LAST_RESULT = res

    outT = np.concatenate([res.results[c]["outT"] for c in range(CORES)],
                          axis=1)          # [256, NP_PAD]
    full = outT.T[:N]                      # [N, 256] in our channel order
    out = np.empty((N, 256), np.float32)
    out[:, gidx] = full
    return out

